# revision 23
# baseline (speedup 1.0000x reference)
"""ConvCaps dynamic-routing kernel for 8 TRN2 NeuronCores (v4).

Strategy (data-parallel over batch B=8, one batch element per core):
  - Grouped 3x3 conv (groups=D=32) in bf16: stationary = im2col patches
    [72, npx], moving = weights [72, 512] per group, PSUM fp32.
    u kept in SBUF as bf16 [px, D, c, d]; no u traffic to DRAM.
  - iter-0 s (uniform routing weights for zero prior) comes free from the
    TensorEngine: a second matmul per group accumulates sum_D u into one
    PSUM bank.
  - Routing einsums on the Vector engine in bf16 2x mode:
      mul: u * w_bcast (broadcast axis kept off the innermost dim)
      reduce: fold-tree of contiguous halves (bf16 tensor_tensor adds run
      2 elem/cycle; tensor_reduce is capped at 1).
  - softmax/squash in fp32; small ops on GpSimd/Scalar to keep Vector on
    the einsums; double-buffered u/x9 for cross-tile overlap.
"""

import numpy as np
from contextlib import ExitStack

import concourse.bacc as bacc
import concourse.bass as bass
import concourse.tile as tile
import concourse.mybir as mybir
from concourse.bass_utils import run_bass_kernel_spmd
from concourse.masks import make_identity

F32 = mybir.dt.float32
BF16 = mybir.dt.bfloat16
AF = mybir.ActivationFunctionType
AX = mybir.AxisListType

B = 8
C_IN, D_IN = 8, 32
C_OUT, D_OUT = 16, 32
KS = 3
H = W = 32
HO = WO = 30
NPX = HO * WO                 # 900 output pixels per batch element
KDIM = C_IN * KS * KS         # 72 = contraction dim of the conv matmul
CD = C_OUT * D_OUT            # 512 out-channels per group
ITERS = 3
P = 128
EPS = 1e-8
ROW_TILES = [(0, 4), (4, 4), (8, 4), (12, 4), (16, 4), (20, 4), (24, 4), (28, 2)]


def _squash_pre(nc, v_dst, s_in, pxs, npx, rp, scale=1.0):
    """Start of squash: v_dst = bf16(s_in) raw (squash factor applied to the
    a-pass RESULT later, since it is constant over c and D), n2 computed,
    sqrt kicked off on the Scalar engine so it overlaps the a-pass."""
    sq, n2, r = rp["sq"], rp["n2"], rp["r"]
    nc.vector.tensor_copy(v_dst[pxs], s_in[pxs])
    nc.vector.tensor_mul(sq[pxs], s_in[pxs], v_dst[pxs])
    # n2[d] = sum_c sq[c,d]: view (d inner-stride-1, c stride-32)
    sqv = sq[pxs].rearrange("p (c d) -> p c d", c=C_OUT).transpose([0, 2, 1])
    nc.vector.reduce_sum(n2[pxs], sqv, axis=AX.X)
    if scale != 1.0:
        nc.vector.tensor_scalar_mul(n2[pxs], n2[pxs], scale * scale)
    nc.vector.tensor_scalar_add(r[pxs], n2[pxs], EPS)
    nc.scalar.activation(r[pxs], r[pxs], AF.Sqrt)


def _squash_post(nc, pxs, rp, scale=1.0):
    """Finish squash: g = scale * n2 / ((1+n2)*sqrt(n2+eps)) into rp["f"]."""
    n2, r, f = rp["n2"], rp["r"], rp["f"]
    nc.vector.tensor_scalar_add(f[pxs], n2[pxs], 1.0)
    nc.vector.tensor_mul(f[pxs], f[pxs], r[pxs])
    nc.vector.reciprocal(f[pxs], f[pxs])
    nc.vector.tensor_mul(f[pxs], f[pxs], n2[pxs])
    if scale != 1.0:
        nc.vector.tensor_scalar_mul(f[pxs], f[pxs], scale)


DH = 16  # d-half size: einsum passes run per d-half to shrink the tmp arena


def _pass_s(nc, tmppool, u_t, c_view, s_dst, pxs, npx):
    """s_dst[P,(c,d)] f32 = sum_D u * c_bcast; per d-half chunks.

    c_view: [P, D, d] (bf16) access pattern (full d)."""
    for h in range(2):
        ds = slice(h * DH, (h + 1) * DH)
        tmp = tmppool.tile([P, D_IN * C_OUT * DH], BF16, tag="tmp", name="tmps")
        tmp2 = tmppool.tile([P, D_IN * C_OUT * DH // 2], BF16, tag="tmp2",
                            name="tmp2s")
        t4 = tmp[pxs].rearrange("p (a b c) -> p a b c", a=D_IN, b=C_OUT)
        cb = c_view[pxs, :, ds].unsqueeze(2).broadcast_to((npx, D_IN, C_OUT, DH))
        nc.vector.tensor_mul(t4, u_t[pxs, :, :, ds], cb)
        # fold over D: 32 -> 1 on contiguous halves (8192..512 elems)
        nc.vector.tensor_add(tmp2[pxs, 0:4096], tmp[pxs, 0:4096], tmp[pxs, 4096:8192])
        nc.vector.tensor_add(tmp[pxs, 0:2048], tmp2[pxs, 0:2048], tmp2[pxs, 2048:4096])
        nc.vector.tensor_add(tmp2[pxs, 0:1024], tmp[pxs, 0:1024], tmp[pxs, 1024:2048])
        nc.vector.tensor_add(tmp[pxs, 0:512], tmp2[pxs, 0:512], tmp2[pxs, 512:1024])
        sd = s_dst[pxs].rearrange("p (c d) -> p c d", c=C_OUT)[:, :, ds]
        sh = tmp[pxs, 0:256].rearrange("p (c d) -> p c d", c=C_OUT)
        nc.vector.tensor_add(sd, sh, tmp[pxs, 256:512].rearrange(
            "p (c d) -> p c d", c=C_OUT))


def _pass_a(nc, tmppool, u_t, v_t, a_dst, pxs, npx):
    """a_dst[P,(D,d)] f32 = sum_c u * v_bcast; per d-half chunks."""
    for h in range(2):
        ds = slice(h * DH, (h + 1) * DH)
        tmp = tmppool.tile([P, D_IN * C_OUT * DH], BF16, tag="tmp", name="tmpa")
        tmp2 = tmppool.tile([P, D_IN * C_OUT * DH // 2], BF16, tag="tmp2",
                            name="tmp2a")
        t4 = tmp[pxs].rearrange("p (a b c) -> p a b c", a=D_IN, b=C_OUT)
        vb = v_t[pxs].rearrange("p (c d) -> p c d", c=C_OUT)[:, :, ds]\
            .unsqueeze(1).broadcast_to((npx, D_IN, C_OUT, DH))
        nc.vector.tensor_mul(t4, u_t[pxs, :, :, ds], vb)
        # fold over c: 16 -> 1; contiguous 128..16-elem runs per D block
        t0 = tmp[pxs].rearrange("p (a x) -> p a x", a=D_IN, x=C_OUT * DH)
        d1 = tmp2[pxs].rearrange("p (a x) -> p a x", a=D_IN, x=C_OUT * DH // 2)
        nc.vector.tensor_add(d1, t0[:, :, 0:128], t0[:, :, 128:256])
        d2 = t0[:, :, 0:64]
        nc.vector.tensor_add(d2, d1[:, :, 0:64], d1[:, :, 64:128])
        d3 = d1[:, :, 0:32]
        nc.vector.tensor_add(d3, d2[:, :, 0:32], d2[:, :, 32:64])
        ad = a_dst[pxs].rearrange("p (a d) -> p a d", a=D_IN)[:, :, ds]
        nc.vector.tensor_add(ad, d3[:, :, 0:DH], d3[:, :, DH:32])


def _body(ctx, tc, xb, wt, b0, c0, out, zero_prior):
    nc = tc.nc
    consts = ctx.enter_context(tc.tile_pool(name="consts", bufs=1))
    x9pool = ctx.enter_context(tc.tile_pool(name="x9pool", bufs=2))
    upool = ctx.enter_context(tc.tile_pool(name="upool", bufs=2))
    tmppool = ctx.enter_context(tc.tile_pool(name="tmppool", bufs=2))
    rpool = ctx.enter_context(tc.tile_pool(name="rpool", bufs=1))
    opool = ctx.enter_context(tc.tile_pool(name="opool", bufs=2))
    psum_c = ctx.enter_context(tc.tile_pool(name="psum_c", bufs=4, space="PSUM"))
    psum_s = ctx.enter_context(tc.tile_pool(name="psum_s", bufs=2, space="PSUM"))
    psum_t = ctx.enter_context(tc.tile_pool(name="psum_t", bufs=2, space="PSUM"))

    w_sb = consts.tile([KDIM, D_IN * CD], BF16)
    nc.sync.dma_start(w_sb[:], wt)
    ident = consts.tile([P, P], F32)
    make_identity(nc, ident)
    b0_sb = consts.tile([P, D_IN * D_OUT], F32)
    nc.sync.dma_start(b0_sb[:], b0)
    if not zero_prior:
        c0_sb = consts.tile([P, D_IN, D_OUT], BF16)
        nc.sync.dma_start(c0_sb[:], c0)

    xbv = xb.rearrange("c (d hw) -> c d hw", d=D_IN)

    # PE warm-up: bf16 dummy matmuls spanning the initial-DMA window so the
    # HAM clock gate is at 8/8 (and stays there) when tile 0's conv issues.
    wdummy = consts.tile([P, P], BF16)
    nc.gpsimd.memset(wdummy[:], 0.0)
    pdump = psum_c.tile([P, CD], F32, tag="pu", name="pdump")
    for _ in range(96):
        nc.tensor.matmul(pdump[:, 0:P], wdummy[:, :], wdummy[:, :],
                         start=True, stop=True)

    dma_engines = [nc.sync, nc.gpsimd, nc.sync, nc.gpsimd]
    n_dma = 0

    for (r0, nr) in ROW_TILES:
        npx = nr * WO
        pxs = slice(0, npx)

        # ---- im2col: per-row 3-dim DMAs (30-wide packed rows)
        x9b = x9pool.tile([KDIM, D_IN, 4, 30], BF16, tag="x9")
        for kh in range(KS):
            for kw in range(KS):
                kk = kh * KS + kw
                for j in range(nr):
                    off = (r0 + kh + j) * W + kw
                    dma_engines[n_dma % 4].dma_start(
                        x9b[kk * C_IN:(kk + 1) * C_IN, :, j, :],
                        xbv[:, :, off:off + 30],
                    )
                    n_dma += 1

        # ---- grouped conv in bf16; ps0 accumulates sum_D u on the PE
        u_t = upool.tile([P, D_IN, C_OUT, D_OUT], BF16, tag="u")
        if zero_prior:
            ps0 = psum_s.tile([P, CD], F32, tag="ps0")
        for g in range(D_IN):
            stat = x9b[:, g, 0:nr, :]
            mov = w_sb[:, g * CD:(g + 1) * CD]
            pu = psum_c.tile([P, CD], F32, tag="pu")
            nc.tensor.matmul(pu[pxs], stat, mov, start=True, stop=True)
            if zero_prior:
                nc.tensor.matmul(ps0[pxs], stat, mov,
                                 start=(g == 0), stop=(g == D_IN - 1))
            udst = u_t[pxs, g].rearrange("p c d -> p (c d)")
            if r0 == 0 and g % 2 == 1:
                # tile 0 is on the critical path: split the psum->u copy
                # chain across Scalar and Vector to halve the startup fill
                nc.vector.tensor_copy(udst, pu[pxs])
            else:
                nc.scalar.copy(udst, pu[pxs])

        # ---- routing state
        rp = {
            "b": rpool.tile([P, D_IN * D_OUT], F32, tag="b", name="rb"),
            "a": rpool.tile([P, D_IN * D_OUT], F32, tag="a", name="ra", bufs=2),
            "e": rpool.tile([P, D_IN, D_OUT], F32, tag="e", name="re", bufs=2),
            "c": rpool.tile([P, D_IN, D_OUT], BF16, tag="c", name="rc", bufs=2),
            "s": rpool.tile([P, CD], F32, tag="s", name="rs"),
            "s0": rpool.tile([P, CD], F32, tag="s0", name="rs0", bufs=2),
            "sq": rpool.tile([P, CD], F32, tag="sq", name="rsq", bufs=2),
            "v": rpool.tile([P, CD], BF16, tag="v", name="rv", bufs=2),
            "z": rpool.tile([P, D_IN], F32, tag="z", name="rz", bufs=2),
            "n2": rpool.tile([P, D_OUT], F32, tag="n2", name="rn2", bufs=2),
            "r": rpool.tile([P, D_OUT], F32, tag="r", name="rr", bufs=2),
            "f": rpool.tile([P, D_OUT], F32, tag="f", name="rf", bufs=2),
        }
        b_t, a_t, s_t, v_t, c_t = rp["b"], rp["a"], rp["s"], rp["v"], rp["c"]

        for it in range(ITERS):
            first, last = it == 0, it == ITERS - 1

            # routing weights c for this iteration
            if first:
                if zero_prior:
                    # s0 straight from the PE accumulation (c uniform = 1/32);
                    # read PSUM directly, 1/32 folded into squash
                    s_cur = ps0
                else:
                    _pass_s(nc, tmppool, u_t, c0_sb, rp["s0"], pxs, npx)
                    s_cur = rp["s0"]
            else:
                # softmax over d: c = exp(b)/Z  (no max-sub; logits are O(1))
                ev = rp["e"]
                nc.scalar.activation(
                    ev[pxs].rearrange("p a b -> p (a b)"), b_t[pxs], AF.Exp)
                nc.vector.reduce_sum(rp["z"][pxs], ev[pxs], axis=AX.X)
                nc.vector.reciprocal(rp["z"][pxs], rp["z"][pxs])
                zb = rp["z"][pxs].unsqueeze(2).broadcast_to((npx, D_IN, D_OUT))
                nc.vector.tensor_mul(c_t[pxs], ev[pxs], zb)
                # s = sum_D c * u
                _pass_s(nc, tmppool, u_t, c_t, s_t, pxs, npx)
                s_cur = s_t

            if last:
                break

            # v = squash(s): run the a-pass on raw s, apply the squash factor
            # g[p,d] (constant over c and D) to the folded result instead --
            # the sqrt/f-chain runs on Scalar concurrently with the a-pass.
            sc = (1.0 / D_IN) if (first and zero_prior) else 1.0
            _squash_pre(nc, v_t, s_cur, pxs, npx, rp, scale=sc)

            # a_raw[D,d] = sum_c u * s_bcast
            _pass_a(nc, tmppool, u_t, v_t, a_t, pxs, npx)

            _squash_post(nc, pxs, rp, scale=sc)
            gb = rp["f"][pxs].unsqueeze(1).broadcast_to((npx, D_IN, D_OUT))
            av = a_t[pxs].rearrange("p (a d) -> p a d", a=D_IN)
            nc.vector.tensor_mul(av, av, gb)
            if first:
                nc.vector.tensor_add(b_t[pxs], b0_sb[pxs], a_t[pxs])
            else:
                nc.vector.tensor_add(b_t[pxs], b_t[pxs], a_t[pxs])

        # ---- write s out as [(c,d), px]: PE transpose in 128-row blocks
        for blk in range(CD // P):
            pt = psum_t.tile([P, 120], F32, tag="pt")
            nc.tensor.transpose(
                pt[:, pxs], s_t[pxs, blk * P:(blk + 1) * P], ident[pxs, pxs])
            ob = opool.tile([P, 120], F32, tag="ob")
            nc.scalar.copy(ob[:, pxs], pt[:, pxs])
            nc.sync.dma_start(
                out[blk * P:(blk + 1) * P, r0 * WO:r0 * WO + npx],
                ob[:, pxs])


_CACHE = {}


def _build(zero_prior: bool):
    key = ("v4", zero_prior)
    if key in _CACHE:
        return _CACHE[key]
    nc = bacc.Bacc("TRN2", target_bir_lowering=False, debug=False,
                   enable_asserts=True, num_devices=B)
    xb = nc.dram_tensor("xb", [C_IN, D_IN * H * W], BF16,
                        kind="ExternalInput").ap()
    wt = nc.dram_tensor("wt", [KDIM, D_IN * CD], BF16,
                        kind="ExternalInput").ap()
    b0 = nc.dram_tensor("b0", [P, D_IN * D_OUT], F32,
                        kind="ExternalInput").ap()
    if not zero_prior:
        c0 = nc.dram_tensor("c0", [P, D_IN, D_OUT], BF16,
                            kind="ExternalInput").ap()
    else:
        c0 = None
    out = nc.dram_tensor("out", [CD, NPX], F32, kind="ExternalOutput").ap()
    with tile.TileContext(nc) as tc:
        with ExitStack() as ctx:
            _body(ctx, tc, xb, wt, b0, c0, out, zero_prior)
    nc.compile()
    _CACHE[key] = nc
    return nc


def _prep_inputs(x, conv_w, prior):
    import ml_dtypes
    bf16 = ml_dtypes.bfloat16
    # weights: rows (D,c,d) x (C,kh,kw) -> [k=(kh,kw,C), (D,c,d)]
    wt = conv_w.reshape(D_IN, C_OUT, D_OUT, C_IN, KS, KS)
    wt = np.ascontiguousarray(wt.transpose(4, 5, 3, 0, 1, 2)).reshape(
        KDIM, D_IN * CD).astype(bf16)
    pb = np.broadcast_to(prior.reshape(D_IN * D_OUT), (P, D_IN * D_OUT))
    b0 = np.ascontiguousarray(pb).astype(np.float32)
    zero_prior = not np.any(prior)
    in_maps = []
    for b in range(B):
        m = {
            "xb": np.ascontiguousarray(x[b].reshape(C_IN, D_IN * H * W)).astype(bf16),
            "wt": wt,
            "b0": b0,
        }
        if not zero_prior:
            pl = prior.reshape(D_IN, D_OUT).astype(np.float64)
            e = np.exp(pl - pl.max(axis=1, keepdims=True))
            c0 = (e / e.sum(axis=1, keepdims=True)).astype(np.float32)
            m["c0"] = np.ascontiguousarray(
                np.broadcast_to(c0, (P, D_IN, D_OUT))).astype(bf16)
        in_maps.append(m)
    return in_maps


def kernel(x, conv_w, prior):
    x = np.asarray(x, dtype=np.float32)
    conv_w = np.asarray(conv_w, dtype=np.float32)
    prior = np.asarray(prior, dtype=np.float32)
    zero_prior = not np.any(prior)
    nc = _build(zero_prior)
    in_maps = _prep_inputs(x, conv_w, prior)
    res = run_bass_kernel_spmd(nc, in_maps, list(range(B)))
    outs = [res.results[b]["out"].reshape(C_OUT, D_OUT, HO, WO)
            for b in range(B)]
    return np.stack(outs, axis=0).astype(np.float32)
